# revision 3
# baseline (speedup 1.0000x reference)
"""NeighborSample Trainium2 kernel, v17: 3-queue byte-balanced schedule.

Input  x:   (8, 64, 64, 192) f32
Output:     (8*64*64, 5, 5, 192) f32 — out[b*4096 + h*64 + w, i, j, c] =
            x[b, h+i-2, w+j-2, c] (zero-padded).

Pure DMA, data-parallel over batch (1 sample per NeuronCore). Input is
zero-padded on the host to (68, 68, 192).

HW model (measured from the v16 ntff profile): the 16 SDMA engines are
a shared ~425 GB/s aggregate ceiling (~26.5 GB/s each); TWO queues
already saturate it (2x210 GB/s), so queue count beyond 2-3 adds
nothing and per-queue rate is just the fair share of 425. Total SDMA
bytes = 78.6 MB stores + 3.7 MB loads = 82.3 MB -> ~194 us floor.
DRAM->DRAM moves count ONCE against the engine ceiling (read+write both
hit HBM, but HBM sustained 640 GB/s in v16, not binding).

v16 lost ~50 us to (a) SWDGE first transfer at ~17 us: the doorbell
only rings after ALL of a dma_start's descriptors are generated
(~7 ns/desc, 2048 descs -> 14 us), (b) SWDGE draining its 15.7 MB by
t=100 us then idling, (c) a c=2 tail store serialized on 2 engines for
the last ~20 us.

v17: balance all three queues at ~27.4 MB and remove all cross-engine
deps:
- gpsimd/SWDGE (Q0): i=0 h0+h1 (c32) and i=4 rows 0..23 of both halves
  (c24), all DRAM->DRAM from padded x — zero dependencies, first
  doorbell early via w-split chunks (512-1024 descs each). 112 rows.
- sync (h0) / scalar (h1) HWDGE rings: loads rows 1-16 (c16), 17-32
  (c16), 33-35 (c3) with dedicated sems, then stores i=1[0:16] (gate
  la1), i=1[16:32] (gate la2), i=4[24:32] c8 (gate la3), i=2 c32,
  i=3 c32. 104 rows + 1.83 MB load each.

DMA fan-out rule (measured): outermost count c splits over n = (largest
divisor of c <= 16) engine slots, c/n consecutive rows per slot. Sem
increments total +16 per DMA; dedicated sem per gating load (a wait can
otherwise be satisfied by another DMA's increments).
"""

import sys

for _p in ("/opt/trn_rl_repo",):
    if _p not in sys.path:
        sys.path.insert(0, _p)

import numpy as np

import concourse.bass as bass
import concourse.mybir as mybir
from concourse.bass_utils import run_bass_kernel_spmd

B = 8
H = W = 64
C = 192
K = 5
PAD = 2
HP = H + 2 * PAD     # 68 padded rows
WP = W + 2 * PAD     # 68 padded cols
ROW = WP * C         # 13056 elems per partition (one padded row)
WIN = K * C          # 960: one (h, w, i) output chunk
OUT_W = K * K * C    # 4800
OUT_H = W * OUT_W    # 307200
HH = H // 2          # 32 output rows per half
HPH = HH + 2 * PAD   # 36 padded rows per half


def _store(eng, out, buf, half, i, h0, cnt):
    """SBUF->DRAM store: shift i, this half's local output rows [h0, h0+cnt)."""
    return eng.dma_start(
        out=bass.AP(
            out,
            (HH * half + h0) * OUT_H + i * WIN,
            [[OUT_H, cnt], [OUT_W, W], [1, WIN]],
        ),
        in_=bass.AP(
            buf, (64 * half + i + h0) * ROW, [[ROW, cnt], [C, W], [1, WIN]]
        ),
    )


def _load(eng, x, buf, half, r0, cnt):
    """Load this half's padded rows [r0, r0+cnt) into partitions."""
    return eng.dma_start(
        out=bass.AP(buf, (64 * half + r0) * ROW, [[ROW, cnt], [1, ROW]]),
        in_=bass.AP(x, (HH * half + r0) * ROW, [[ROW, cnt], [1, ROW]]),
    )


def _dram_store(eng, x, out, half, i, h0, cnt, w0, wcnt):
    """Shift i direct from padded x in DRAM — no SBUF, no deps.

    Covers output rows [h0, h0+cnt) of this half, w in [w0, w0+wcnt).
    """
    return eng.dma_start(
        out=bass.AP(
            out,
            (HH * half + h0) * OUT_H + i * WIN + w0 * OUT_W,
            [[OUT_H, cnt], [OUT_W, wcnt], [1, WIN]],
        ),
        in_=bass.AP(
            x,
            (HH * half + h0 + i) * ROW + w0 * C,
            [[ROW, cnt], [C, wcnt], [1, WIN]],
        ),
    )


def _emit_ring(eng, x, out, buf, la1, la2, la3, dsem, half):
    _load(eng, x, buf, half, 1, 16).then_inc(la1, 16)
    _load(eng, x, buf, half, 17, 16).then_inc(la2, 16)
    _load(eng, x, buf, half, 33, 3).then_inc(la3, 16)
    eng.wait_ge(la1, 16)
    _store(eng, out, buf, half, 1, 0, 16).then_inc(dsem, 16)
    eng.wait_ge(la2, 16)
    _store(eng, out, buf, half, 1, 16, 16).then_inc(dsem, 16)
    eng.wait_ge(la3, 16)
    _store(eng, out, buf, half, 4, 24, 8).then_inc(dsem, 16)
    _store(eng, out, buf, half, 2, 0, 32).then_inc(dsem, 16)
    _store(eng, out, buf, half, 3, 0, 32).then_inc(dsem, 16)
    eng.wait_ge(dsem, 16 * 5)


def build_nc() -> bass.Bass:
    nc = bass.Bass()
    x = nc.declare_dram_parameter("x", [HP, WP, C], mybir.dt.float32, isOutput=False)
    out = nc.declare_dram_parameter(
        "out", [H, W, K, K, C], mybir.dt.float32, isOutput=True
    )

    with (
        nc.Block() as block,
        nc.semaphore("la1") as la1,
        nc.semaphore("la2") as la2,
        nc.semaphore("la3") as la3,
        nc.semaphore("d_a") as d_a,
        nc.semaphore("lb1") as lb1,
        nc.semaphore("lb2") as lb2,
        nc.semaphore("lb3") as lb3,
        nc.semaphore("d_b") as d_b,
        nc.semaphore("d_g") as d_g,
        nc.sbuf_tensor("buf", [128, ROW], mybir.dt.float32) as buf,
    ):

        @block.sync
        def _(sync):
            _emit_ring(sync, x, out, buf, la1, la2, la3, d_a, 0)

        @block.scalar
        def _(scalar):
            _emit_ring(scalar, x, out, buf, lb1, lb2, lb3, d_b, 1)

        @block.gpsimd
        def _(gpsimd):
            n = 0
            # i=0 h0: w-quarters first for an early doorbell, then a half
            for w0, wc in ((0, 16), (16, 16), (32, 32)):
                _dram_store(gpsimd, x, out, 0, 0, 0, 32, w0, wc).then_inc(d_g, 16)
                n += 1
            for w0, wc in ((0, 32), (32, 32)):
                _dram_store(gpsimd, x, out, 1, 0, 0, 32, w0, wc).then_inc(d_g, 16)
                n += 1
            for half in (0, 1):
                for w0, wc in ((0, 32), (32, 32)):
                    _dram_store(gpsimd, x, out, half, 4, 0, 24, w0, wc).then_inc(
                        d_g, 16
                    )
                    n += 1
            gpsimd.wait_ge(d_g, 16 * n)

    return nc


_NC_CACHE = None


def prep_in_maps(x):
    xp = np.zeros((B, HP, WP, C), dtype=np.float32)
    xp[:, PAD : PAD + H, PAD : PAD + W, :] = x
    return [{"x": np.ascontiguousarray(xp[i])} for i in range(B)]


def kernel(x) -> np.ndarray:
    global _NC_CACHE
    x = np.asarray(x, dtype=np.float32)
    assert x.shape == (B, H, W, C), x.shape
    if _NC_CACHE is None:
        _NC_CACHE = build_nc()
    in_maps = prep_in_maps(x)
    res = run_bass_kernel_spmd(_NC_CACHE, in_maps, list(range(B)))
    outs = [res.results[i]["out"].reshape(H * W, K, K, C) for i in range(B)]
    return np.concatenate(outs, axis=0)


# revision 4
# speedup vs baseline: 1.4279x; 1.4279x over previous
"""NeighborSample Trainium2 kernel, v18: all-c32 3-queue balanced schedule.

Input  x:   (8, 64, 64, 192) f32
Output:     (8*64*64, 5, 5, 192) f32 — out[b*4096 + h*64 + w, i, j, c] =
            x[b, h+i-2, w+j-2, c] (zero-padded).

Pure DMA, data-parallel over batch (1 sample per NeuronCore). Input is
zero-padded on the host to (68, 68, 192).

HW model (measured via ntff profiles of v16/v17):
- The 16 SDMA engines are a shared ~425 GB/s ceiling (~26.5 GB/s
  each, slice-rate 25.7 GB/s for 3840 B store descriptors). Queue
  count beyond 2 adds nothing; per-queue rate is a demand-weighted
  share (SWDGE's 2-desc packets get a 2:1 round-robin share).
- SWDGE (gpsimd, Q0) is desc-gen limited to ~55 desc/us ≈ 212 GB/s,
  and a dma_start's doorbell only rings after ALL its descriptors are
  generated (first transfer ~17 us for a 2048-desc DMA).
- DRAM->DRAM counts once against the engine ceiling; HBM sustained
  640 GB/s total in v16, not binding.
- Stores with <16 engine slots (c8/c24/the old c30+c2 tail) strand
  engine capacity — v17 measured a ~20-30% aggregate loss in phases
  dominated by such DMAs. Everything here is c32 (16 slots x 2 rows).

Total SDMA bytes 82.2 MB -> ~194 us floor + ~15 us ramp/preamble.
v16 = 243.9 us lost ~25 us to a c=2 2-engine tail and ~20 us to Q0
idling after t=100 us (it only had 15.7 MB).

v18 layout (10 c32 stores, 2 halves x 5 shifts):
- gpsimd/SWDGE: i=0 and i=4 for both halves, DRAM->DRAM from padded x
  (zero deps, 31.4 MB ≈ its gen-limited fair share over the run).
- sync (h0) / scalar (h1): load rows 1-32 (c32) + rows 33-34 (c2,
  dedicated sems), then stores i=1 (gate la1), i=2, i=3 (gate la3).
  25.4 MB each. No cross-engine dependencies anywhere.

DMA fan-out rule (measured): outermost count c splits over n = (largest
divisor of c <= 16) engine slots, c/n consecutive rows per slot. Sem
increments total +16 per DMA; a dedicated sem per gating load (a wait
can otherwise be satisfied by another DMA's increments).
"""

import sys

for _p in ("/opt/trn_rl_repo",):
    if _p not in sys.path:
        sys.path.insert(0, _p)

import numpy as np

import concourse.bass as bass
import concourse.mybir as mybir
from concourse.bass_utils import run_bass_kernel_spmd

B = 8
H = W = 64
C = 192
K = 5
PAD = 2
HP = H + 2 * PAD     # 68 padded rows
WP = W + 2 * PAD     # 68 padded cols
ROW = WP * C         # 13056 elems per partition (one padded row)
WIN = K * C          # 960: one (h, w, i) output chunk
OUT_W = K * K * C    # 4800
OUT_H = W * OUT_W    # 307200
HH = H // 2          # 32 output rows per half


def _store(eng, out, buf, half, i, h0, cnt):
    """SBUF->DRAM store: shift i, this half's local output rows [h0, h0+cnt)."""
    return eng.dma_start(
        out=bass.AP(
            out,
            (HH * half + h0) * OUT_H + i * WIN,
            [[OUT_H, cnt], [OUT_W, W], [1, WIN]],
        ),
        in_=bass.AP(
            buf, (64 * half + i + h0) * ROW, [[ROW, cnt], [C, W], [1, WIN]]
        ),
    )


def _load(eng, x, buf, half, r0, cnt):
    """Load this half's padded rows [r0, r0+cnt) into partitions."""
    return eng.dma_start(
        out=bass.AP(buf, (64 * half + r0) * ROW, [[ROW, cnt], [1, ROW]]),
        in_=bass.AP(x, (HH * half + r0) * ROW, [[ROW, cnt], [1, ROW]]),
    )


def _dram_store(eng, x, out, half, i):
    """Full-half shift i direct from padded x in DRAM — no SBUF, no deps."""
    return eng.dma_start(
        out=bass.AP(
            out,
            (HH * half) * OUT_H + i * WIN,
            [[OUT_H, HH], [OUT_W, W], [1, WIN]],
        ),
        in_=bass.AP(
            x,
            (HH * half + i) * ROW,
            [[ROW, HH], [C, W], [1, WIN]],
        ),
    )


def _emit_ring(eng, x, out, buf, la1, la3, dsem, half):
    _load(eng, x, buf, half, 1, 32).then_inc(la1, 16)
    _load(eng, x, buf, half, 33, 2).then_inc(la3, 16)
    eng.wait_ge(la1, 16)
    _store(eng, out, buf, half, 1, 0, HH).then_inc(dsem, 16)
    eng.wait_ge(la3, 16)
    _store(eng, out, buf, half, 2, 0, HH).then_inc(dsem, 16)
    _store(eng, out, buf, half, 3, 0, HH).then_inc(dsem, 16)
    eng.wait_ge(dsem, 16 * 3)


def build_nc() -> bass.Bass:
    nc = bass.Bass()
    x = nc.declare_dram_parameter("x", [HP, WP, C], mybir.dt.float32, isOutput=False)
    out = nc.declare_dram_parameter(
        "out", [H, W, K, K, C], mybir.dt.float32, isOutput=True
    )

    with (
        nc.Block() as block,
        nc.semaphore("la1") as la1,
        nc.semaphore("la3") as la3,
        nc.semaphore("d_a") as d_a,
        nc.semaphore("lb1") as lb1,
        nc.semaphore("lb3") as lb3,
        nc.semaphore("d_b") as d_b,
        nc.semaphore("d_g") as d_g,
        nc.sbuf_tensor("buf", [128, ROW], mybir.dt.float32) as buf,
    ):

        @block.sync
        def _(sync):
            _emit_ring(sync, x, out, buf, la1, la3, d_a, 0)

        @block.scalar
        def _(scalar):
            _emit_ring(scalar, x, out, buf, lb1, lb3, d_b, 1)

        @block.gpsimd
        def _(gpsimd):
            _dram_store(gpsimd, x, out, 0, 0).then_inc(d_g, 16)
            _dram_store(gpsimd, x, out, 1, 0).then_inc(d_g, 16)
            _dram_store(gpsimd, x, out, 0, 4).then_inc(d_g, 16)
            _dram_store(gpsimd, x, out, 1, 4).then_inc(d_g, 16)
            gpsimd.wait_ge(d_g, 16 * 4)

    return nc


_NC_CACHE = None


def prep_in_maps(x):
    xp = np.zeros((B, HP, WP, C), dtype=np.float32)
    xp[:, PAD : PAD + H, PAD : PAD + W, :] = x
    return [{"x": np.ascontiguousarray(xp[i])} for i in range(B)]


def kernel(x) -> np.ndarray:
    global _NC_CACHE
    x = np.asarray(x, dtype=np.float32)
    assert x.shape == (B, H, W, C), x.shape
    if _NC_CACHE is None:
        _NC_CACHE = build_nc()
    in_maps = prep_in_maps(x)
    res = run_bass_kernel_spmd(_NC_CACHE, in_maps, list(range(B)))
    outs = [res.results[i]["out"].reshape(H * W, K, K, C) for i in range(B)]
    return np.concatenate(outs, axis=0)
